# revision 19
# baseline (speedup 1.0000x reference)
"""Trainium2 Bass kernel for nn_MultiHeadAttention (B=4, C=256, L=2048, H=8, D=32).

Sharding: 8 cores = (batch b = core//2) x (channel/head half hf = core%2).
Each core computes 4 heads (128 of 256 channels) for one batch:
  - conv1x1+BN projections folded on host into W_eff (bf16) + per-position
    bias maps (W_eff @ pe + b_eff), 1/sqrt(D) folded into W_q.
  - scores computed TRANSPOSED (s^T[m,l]) with 4-way ROW-TILED matmuls
    (K=32 per head, tile_position=(32h,0)) into a 2-bank PSUM tile per
    head-pair.
  - exp on ACT (native Exp) and DVE (Schraudolph int16->bf16 bit trick),
    running concurrently, PSUM -> SBUF bf16 p^T.
  - a_unnorm = sum_m p~^T v and rowsum via 4-way COL-TILED accumulating
    matmuls (tile_position=(0,32h)); rowsum uses a ones[128,32] lhsT which
    also broadcasts the sum across each head's 32 partitions.
  - a_norm = a_unnorm * recip(rowsum); partial out-proj wo_half @ a_norm
    -> DRAM. Host sums the two halves + residual q + bo.
"""

import numpy as np
import ml_dtypes

B, CQ, CVK, L, H = 4, 256, 256, 2048, 8
D = CVK // H  # 32
EPS = 1e-5
P = 128
N_CORES = 8
LT = 512          # l-tile width
NLT = L // LT     # 4
NMT = L // P      # 16 m-tiles

# Schraudolph exp in bf16-bit domain: bits_i16 = s*SCH_A + SCH_B (converted
# to int16), bitcast to bf16. SCH_A = 2^7/ln2. SCH_B tuned for balanced
# multiplicative error (constant factor cancels in softmax).
SCH_A = 184.6650558352966
SCH_B = 16256.0 - 7.34
# m-tile indices whose heads-2/3 half goes to DVE instead of ACT.
DVE_SET = frozenset(j for j in range(NMT) if j not in (3, 9, 14))

PROFILE = False
LAST_RESULT = None

_compiled = {}


def _build(repeat=1, loop_iters=1, variant="full"):
    key = (repeat, loop_iters, variant)
    if key in _compiled:
        return _compiled[key]

    import concourse.mybir as mybir
    import concourse.tile as tile
    from concourse import bacc

    f32 = mybir.dt.float32
    bf16 = mybir.dt.bfloat16
    i16 = mybir.dt.int16
    Alu = mybir.AluOpType
    Act = mybir.ActivationFunctionType

    nc = bacc.Bacc("TRN2", target_bir_lowering=False, debug=False,
                   enable_asserts=False)

    def din(name, shape, dt=bf16):
        return nc.dram_tensor(name, list(shape), dt, kind="ExternalInput").ap()

    xq_d = din("xq", (P, 2, L))
    xk_d = din("xk", (P, 2, L))
    xv_d = din("xv", (P, 2, L))
    wq_d = din("wq", (P, 2, P))
    wk_d = din("wk", (P, 2, P))
    wv_d = din("wv", (P, 2, P))
    wo_d = din("wo", (P, 2 * P))
    bqm_d = din("bqm", (P, L))
    bkm_d = din("bkm", (P, L))
    bvm_d = din("bvm", (P, NMT, P))
    out_d = nc.dram_tensor("out", [CQ, L], f32, kind="ExternalOutput").ap()

    do_scores = variant in ("full", "allact", "alldve", "nors", "noavrs")
    do_rs = variant in ("full", "allact", "alldve")
    do_av = variant != "noavrs"

    with tile.TileContext(nc) as tc:
        with tc.tile_pool(name="sbuf", bufs=1) as sb, \
             tc.tile_pool(name="ptp", bufs=4) as ptp, \
             tc.tile_pool(name="epi", bufs=2) as epi, \
             tc.tile_pool(name="avp", bufs=1, space="PSUM") as avp, \
             tc.tile_pool(name="rsp", bufs=1, space="PSUM") as rsp, \
             tc.tile_pool(name="sp", bufs=3, space="PSUM") as spp:

          def body():
            # ---- loads (all via HWDGE) ----
            xq_s = sb.tile([P, 2, L], bf16, name="xq_s", tag="xq_s")
            xk_s = sb.tile([P, 2, L], bf16, name="xk_s", tag="xk_s")
            xv_s = sb.tile([P, 2, L], bf16, name="xv_s", tag="xv_s")
            wq_s = sb.tile([P, 2, P], bf16, name="wq_s", tag="wq_s")
            wk_s = sb.tile([P, 2, P], bf16, name="wk_s", tag="wk_s")
            wv_s = sb.tile([P, 2, P], bf16, name="wv_s", tag="wv_s")
            wo_s = sb.tile([P, 2 * P], bf16, name="wo_s", tag="wo_s")
            bqm_s = sb.tile([P, L], bf16, name="bqm_s", tag="bqm_s")
            bkm_s = sb.tile([P, L], bf16, name="bkm_s", tag="bkm_s")
            bvm_s = sb.tile([P, NMT, P], bf16, name="bvm_s", tag="bvm_s")
            for s_t, d_t in ((wq_s, wq_d), (wk_s, wk_d), (wv_s, wv_d),
                             (wo_s, wo_d), (xk_s, xk_d), (xq_s, xq_d),
                             (xv_s, xv_d), (bqm_s, bqm_d), (bkm_s, bkm_d),
                             (bvm_s, bvm_d)):
                nc.sync.dma_start(s_t, d_t)

            ones_s = sb.tile([P, D], bf16, name="ones_s", tag="ones_s")
            nc.vector.memset(ones_s, 1.0)

            if variant == "dmaonly":
                op_sb = epi.tile([P, LT], f32, name="op_sb", tag="op_sb")
                nc.vector.tensor_copy(op_sb, xq_s[:, 0, :LT])
                nc.vector.tensor_add(op_sb, op_sb, bqm_s[:, :LT])
                nc.vector.tensor_add(op_sb, op_sb, bkm_s[:, :LT])
                nc.vector.tensor_add(op_sb, op_sb,
                                     bvm_s[:, 0:4, :].rearrange("p a b -> p (a b)"))
                nc.vector.tensor_add(op_sb, op_sb, xk_s[:, 0, :LT])
                nc.vector.tensor_add(op_sb, op_sb, xv_s[:, 0, :LT])
                nc.vector.tensor_add(op_sb[:, :P], op_sb[:, :P], wq_s[:, 0, :P])
                nc.vector.tensor_add(op_sb[:, :P], op_sb[:, :P], wk_s[:, 0, :P])
                nc.vector.tensor_add(op_sb[:, :P], op_sb[:, :P], wv_s[:, 0, :P])
                nc.vector.tensor_add(op_sb[:, :2 * P], op_sb[:, :2 * P], wo_s)
                for mt in range(2):
                    for lt2 in range(NLT):
                        nc.sync.dma_start(
                            out_d[mt * P:(mt + 1) * P,
                                  lt2 * LT:(lt2 + 1) * LT], op_sb)
                return

            # projected activations
            qp_s = sb.tile([P, L], bf16, name="qp_s", tag="qp_s")
            kp_s = sb.tile([P, L], bf16, name="kp_s", tag="kp_s")
            vpt_s = sb.tile([P, NMT, P], bf16, name="vpt_s", tag="vpt_s")

            # ---- projections (psum shared with attention's s_t slots) ----
            for t in range(NLT):
                for (w_s, x_s, bm_s, dst) in ((wk_s, xk_s, bkm_s, kp_s),
                                              (wq_s, xq_s, bqm_s, qp_s)):
                    ps = spp.tile([P, LT], f32, name="ps", tag="s_t")
                    nc.tensor.matmul(ps, w_s[:, 0, :],
                                     x_s[:, 0, t * LT:(t + 1) * LT],
                                     start=True, stop=False)
                    nc.tensor.matmul(ps, w_s[:, 1, :],
                                     x_s[:, 1, t * LT:(t + 1) * LT],
                                     start=False, stop=True)
                    nc.vector.tensor_tensor(
                        dst[:, t * LT:(t + 1) * LT], ps,
                        bm_s[:, t * LT:(t + 1) * LT], Alu.add)
                # v projection, transposed output: [l-chunk, ch]
                psv = spp.tile([P, LT], f32, name="psv", tag="s_t")
                for c4 in range(4):
                    c = 4 * t + c4
                    nc.tensor.matmul(psv[:, c4 * P:(c4 + 1) * P],
                                     xv_s[:, 0, c * P:(c + 1) * P],
                                     wv_s[:, 0, :], start=True, stop=False)
                    nc.tensor.matmul(psv[:, c4 * P:(c4 + 1) * P],
                                     xv_s[:, 1, c * P:(c + 1) * P],
                                     wv_s[:, 1, :], start=False, stop=True)
                nc.vector.tensor_tensor(
                    vpt_s[:, 4 * t:4 * t + 4, :], psv,
                    bvm_s[:, 4 * t:4 * t + 4, :], Alu.add)

            if variant in ("actonly", "dveonly"):
                for lt in range(NLT):
                    av_ps = avp.tile([P, LT], f32, name="av_ps", tag="av_ps")
                    s_ps = spp.tile([P, 2 * LT], f32, name="s_ps", tag="s_t")
                    for hh in range(2):
                        nc.tensor.matmul(
                            s_ps[:, hh * LT:(hh + 1) * LT],
                            kp_s[hh * D:(hh + 1) * D, :P],
                            qp_s[hh * D:(hh + 1) * D, lt * LT:(lt + 1) * LT],
                            start=True, stop=True, tile_position=(hh * D, 0))
                    for j in range(NMT):
                        for g in range(2):
                            pt = ptp.tile([P, 2, LT], bf16, name=f"pt{g}",
                                          tag=f"pt{g}")
                            pt_flat = pt.rearrange("p a b -> p (a b)")
                            if variant == "actonly":
                                nc.scalar.activation(pt_flat, s_ps, Act.Exp)
                            else:
                                nc.vector.tensor_scalar(
                                    pt_flat.bitcast(i16), s_ps,
                                    SCH_A, SCH_B, Alu.mult, Alu.add)
                            nc.tensor.matmul(
                                av_ps[:D, :], ones_s, pt[:, 0, :],
                                start=True, stop=True,
                                tile_position=(0, 0))
                    op_sb = epi.tile([P, LT], f32, name="op_sb", tag="op_sb")
                    nc.vector.tensor_copy(op_sb, s_ps[:, :LT])
                    nc.sync.dma_start(
                        out_d[:P, lt * LT:(lt + 1) * LT], op_sb)
                return

            # ---- attention ----
            for lt in range(NLT):
                av_ps = avp.tile([P, LT], f32, name="av_ps", tag="av_ps")
                rs_ps = rsp.tile([P, LT], f32, name="rs_ps", tag="rs_ps")
                for j in range(NMT):
                    pts = [ptp.tile([P, 2, LT], bf16, name=f"pt{g}",
                                    tag=f"pt{g}") for g in range(2)]
                    if do_scores:
                        for g in range(2):  # head pairs (0,1) and (2,3)
                            s_ps = spp.tile([P, 2 * LT], f32, name="s_ps",
                                            tag="s_t")
                            for hh in range(2):
                                h = 2 * g + hh
                                nc.tensor.matmul(
                                    s_ps[:, hh * LT:(hh + 1) * LT],
                                    kp_s[h * D:(h + 1) * D, j * P:(j + 1) * P],
                                    qp_s[h * D:(h + 1) * D, lt * LT:(lt + 1) * LT],
                                    start=True, stop=True,
                                    tile_position=(h * D, 0))
                            pt_flat = pts[g].rearrange("p a b -> p (a b)")
                            use_dve = (variant == "alldve" or
                                       (variant in ("full", "nors") and
                                        g == 1 and j in DVE_SET))
                            if variant == "allact":
                                use_dve = False
                            if use_dve:
                                nc.vector.tensor_scalar(
                                    pt_flat.bitcast(i16), s_ps,
                                    SCH_A, SCH_B, Alu.mult, Alu.add)
                            else:
                                nc.scalar.activation(pt_flat, s_ps, Act.Exp)
                    else:
                        for g in range(2):
                            nc.vector.memset(pts[g], 1.0)
                    if do_av:
                        for h in range(4):
                            nc.tensor.matmul(
                                av_ps[h * D:(h + 1) * D, :],
                                vpt_s[:, j, h * D:(h + 1) * D],
                                pts[h // 2][:, h % 2, :],
                                start=(j == 0), stop=(j == NMT - 1),
                                tile_position=(0, h * D))
                    elif do_scores:
                        # keep exps live: cheap consumers of both pt tiles
                        for h in (0, 2):
                            nc.tensor.matmul(
                                av_ps[h * D:(h + 1) * D, :],
                                vpt_s[:, j, h * D:(h + 1) * D],
                                pts[h // 2][:, h % 2, :],
                                start=(j == 0), stop=(j == NMT - 1),
                                tile_position=(0, h * D))
                    if do_rs:
                        for h in range(4):
                            nc.tensor.matmul(
                                rs_ps[h * D:(h + 1) * D, :],
                                ones_s, pts[h // 2][:, h % 2, :],
                                start=(j == 0), stop=(j == NMT - 1),
                                tile_position=(0, h * D))
                recip = epi.tile([P, LT], f32, name="recip", tag="recip")
                nc.vector.reciprocal_approx_fast(
                    recip, rs_ps if do_rs else av_ps)
                anorm = epi.tile([P, LT], bf16, name="anorm", tag="anorm")
                nc.vector.tensor_tensor(anorm, av_ps, recip, Alu.mult)
                for mt in range(2):
                    op_ps = spp.tile([P, LT], f32, name="op_ps", tag="s_t")
                    nc.tensor.matmul(op_ps, wo_s[:, mt * P:(mt + 1) * P],
                                     anorm, start=True, stop=True)
                    op_sb = epi.tile([P, LT], f32, name="op_sb", tag="op_sb")
                    nc.vector.tensor_copy(op_sb, op_ps)
                    nc.sync.dma_start(
                        out_d[mt * P:(mt + 1) * P, lt * LT:(lt + 1) * LT],
                        op_sb)

          if loop_iters > 1:
              with tc.For_i(0, loop_iters, 1):
                  body()
          else:
              for _rep in range(repeat):
                  body()

    nc.compile()
    _compiled[key] = nc
    return nc


def _prep_core_inputs(inputs):
    """Host-side folding; returns list of 8 per-core input dicts (bf16)."""
    f8 = np.float64
    bf = ml_dtypes.bfloat16
    q = np.asarray(inputs['q'], np.float32)
    k = np.asarray(inputs['k'], np.float32)
    v = np.asarray(inputs['v'], np.float32)
    pe_q = np.asarray(inputs['pe_q'], f8)
    pe_vk = np.asarray(inputs['pe_vk'], f8)

    def fold(w, b, g, beta, m, var):
        scale = np.asarray(g, f8) / np.sqrt(np.asarray(var, f8) + EPS)
        w_eff = scale[:, None] * np.asarray(w, f8)
        b_eff = (np.asarray(b, f8) - np.asarray(m, f8)) * scale + np.asarray(beta, f8)
        return w_eff, b_eff

    wq_e, bq_e = fold(inputs['wq'], inputs['bq'], inputs['gq'],
                      inputs['betaq'], inputs['mq'], inputs['varq'])
    wk_e, bk_e = fold(inputs['wk'], inputs['bk'], inputs['gk'],
                      inputs['betak'], inputs['mk'], inputs['vark'])
    wv_e, bv_e = fold(inputs['wv'], inputs['bv'], inputs['gv'],
                      inputs['betav'], inputs['mv'], inputs['varv'])
    sc = 1.0 / np.sqrt(np.float64(D))
    wq_e = wq_e * sc
    bq_e = bq_e * sc
    wo = np.asarray(inputs['wo'], f8)

    def wlayout(w_half):
        # [256 in, 128 out] -> [128 p, 2 ko, 128 m]
        return np.ascontiguousarray(
            w_half.T.reshape(2, P, P).transpose(1, 0, 2)).astype(bf)

    def xlayout(x):
        return np.ascontiguousarray(
            x.reshape(2, P, L).transpose(1, 0, 2)).astype(bf)

    half_data = []
    for hf in range(2):
        r = slice(hf * P, (hf + 1) * P)
        wq_h, wk_h, wv_h = wq_e[r], wk_e[r], wv_e[r]
        bqm = wq_h @ pe_q + bq_e[r][:, None]           # [128, L]
        bkm = wk_h @ pe_vk + bk_e[r][:, None]
        bvm = wv_h @ pe_vk + bv_e[r][:, None]
        bvmT = np.ascontiguousarray(
            bvm.T.reshape(NMT, P, P).transpose(1, 0, 2))  # [p, chunk, ch]
        half_data.append(dict(
            wq=wlayout(wq_h), wk=wlayout(wk_h), wv=wlayout(wv_h),
            wo=np.ascontiguousarray(wo[:, r].T).astype(bf),
            bqm=bqm.astype(bf), bkm=bkm.astype(bf), bvm=bvmT.astype(bf),
        ))

    in_maps = []
    for core in range(N_CORES):
        b, hf = core // 2, core % 2
        m = dict(half_data[hf])
        m['xq'] = xlayout(q[b])
        m['xk'] = xlayout(k[b])
        m['xv'] = xlayout(v[b])
        in_maps.append(m)
    return in_maps


def kernel(**inputs):
    global LAST_RESULT
    from concourse.bass_utils import run_bass_kernel_spmd

    nc = _build()
    in_maps = _prep_core_inputs(inputs)
    res = run_bass_kernel_spmd(nc, in_maps, core_ids=list(range(N_CORES)),
                               trace=PROFILE)
    LAST_RESULT = res

    q = np.asarray(inputs['q'], np.float32)
    bo = np.asarray(inputs['bo'], np.float32)
    out = np.empty((B, CQ, L), np.float32)
    for b in range(B):
        out[b] = (q[b] + bo[:, None]
                  + res.results[2 * b]['out'] + res.results[2 * b + 1]['out'])
    return out


# revision 21
# speedup vs baseline: 1.9508x; 1.9508x over previous
"""Trainium2 Bass kernel for nn_MultiHeadAttention (B=4, C=256, L=2048, H=8, D=32).

Sharding: 8 cores = (batch b = core//2) x (channel/head half hf = core%2).
Each core computes 4 heads (128 of 256 channels) for one batch:
  - conv1x1+BN projections folded on host into W_eff (bf16) + per-position
    bias maps (W_eff @ pe + b_eff), 1/sqrt(D) folded into W_q.
  - scores computed TRANSPOSED (s^T[m,l]) with 4-way ROW-TILED matmuls
    (K=32 per head, tile_position=(32h,0)) into a 2-bank PSUM tile per
    head-pair.
  - exp on ACT (native Exp) and DVE (Schraudolph int16->bf16 bit trick),
    running concurrently, PSUM -> SBUF bf16 p^T.
  - a_unnorm = sum_m p~^T v and rowsum via 4-way COL-TILED accumulating
    matmuls (tile_position=(0,32h)); rowsum uses a ones[128,32] lhsT which
    also broadcasts the sum across each head's 32 partitions.
  - a_norm = a_unnorm * recip(rowsum); partial out-proj wo_half @ a_norm
    -> DRAM. Host sums the two halves + residual q + bo.
"""

import numpy as np
import ml_dtypes

B, CQ, CVK, L, H = 4, 256, 256, 2048, 8
D = CVK // H  # 32
EPS = 1e-5
P = 128
N_CORES = 8
LT = 512          # l-tile width
NLT = L // LT     # 4
NMT = L // P      # 16 m-tiles

# Schraudolph exp in bf16-bit domain: bits_i16 = s*SCH_A + SCH_B (converted
# to int16), bitcast to bf16. SCH_A = 2^7/ln2. SCH_B tuned for balanced
# multiplicative error (constant factor cancels in softmax).
SCH_A = 184.6650558352966
SCH_B = 16256.0 - 7.34
# m-tile indices whose heads-2/3 half goes to DVE instead of ACT.
DVE_SET = frozenset(j for j in range(NMT) if j not in (3, 9, 14))

PROFILE = False
LAST_RESULT = None

_compiled = {}


def _build(repeat=1, loop_iters=1, variant="full"):
    key = (repeat, loop_iters, variant)
    if key in _compiled:
        return _compiled[key]

    import concourse.mybir as mybir
    import concourse.tile as tile
    from concourse import bacc

    f32 = mybir.dt.float32
    bf16 = mybir.dt.bfloat16
    i16 = mybir.dt.int16
    Alu = mybir.AluOpType
    Act = mybir.ActivationFunctionType

    nc = bacc.Bacc("TRN2", target_bir_lowering=False, debug=False,
                   enable_asserts=False)

    def din(name, shape, dt=bf16):
        return nc.dram_tensor(name, list(shape), dt, kind="ExternalInput").ap()

    xq_d = din("xq", (P, 2, L))
    xk_d = din("xk", (P, 2, L))
    xv_d = din("xv", (P, 2, L))
    wq_d = din("wq", (P, 2, P))
    wk_d = din("wk", (P, 2, P))
    wv_d = din("wv", (P, 2, P))
    wo_d = din("wo", (P, 2 * P))
    bqm_d = din("bqm", (P, L))
    bkm_d = din("bkm", (P, L))
    bvm_d = din("bvm", (P, NMT, P))
    out_d = nc.dram_tensor("out", [CQ, L], f32, kind="ExternalOutput").ap()

    do_scores = variant in ("full", "allact", "alldve", "nors", "noavrs")
    do_rs = variant in ("full", "allact", "alldve")
    do_av = variant != "noavrs"

    with tile.TileContext(nc) as tc:
        with tc.tile_pool(name="sbuf", bufs=1) as sb, \
             tc.tile_pool(name="ptp", bufs=4) as ptp, \
             tc.tile_pool(name="epi", bufs=2) as epi, \
             tc.tile_pool(name="avp", bufs=1, space="PSUM") as avp, \
             tc.tile_pool(name="rsp", bufs=1, space="PSUM") as rsp, \
             tc.tile_pool(name="sp", bufs=3, space="PSUM") as spp:

          def body():
            # ---- loads (all via HWDGE) ----
            xq_s = sb.tile([P, 2, L], bf16, name="xq_s", tag="xq_s")
            xk_s = sb.tile([P, 2, L], bf16, name="xk_s", tag="xk_s")
            xv_s = sb.tile([P, 2, L], bf16, name="xv_s", tag="xv_s")
            wq_s = sb.tile([P, 2, P], bf16, name="wq_s", tag="wq_s")
            wk_s = sb.tile([P, 2, P], bf16, name="wk_s", tag="wk_s")
            wv_s = sb.tile([P, 2, P], bf16, name="wv_s", tag="wv_s")
            wo_s = sb.tile([P, 2 * P], bf16, name="wo_s", tag="wo_s")
            bqm_s = sb.tile([P, L], bf16, name="bqm_s", tag="bqm_s")
            bkm_s = sb.tile([P, L], bf16, name="bkm_s", tag="bkm_s")
            bvm_s = sb.tile([P, NMT, P], bf16, name="bvm_s", tag="bvm_s")
            for s_t, d_t in ((wq_s, wq_d), (wk_s, wk_d), (wv_s, wv_d),
                             (wo_s, wo_d), (xk_s, xk_d), (xq_s, xq_d),
                             (xv_s, xv_d), (bqm_s, bqm_d), (bkm_s, bkm_d),
                             (bvm_s, bvm_d)):
                nc.sync.dma_start(s_t, d_t)

            ones_s = sb.tile([P, D], bf16, name="ones_s", tag="ones_s")
            nc.vector.memset(ones_s, 1.0)

            if variant == "dmaonly":
                op_sb = epi.tile([P, LT], f32, name="op_sb", tag="op_sb")
                nc.vector.tensor_copy(op_sb, xq_s[:, 0, :LT])
                nc.vector.tensor_add(op_sb, op_sb, bqm_s[:, :LT])
                nc.vector.tensor_add(op_sb, op_sb, bkm_s[:, :LT])
                nc.vector.tensor_add(op_sb, op_sb,
                                     bvm_s[:, 0:4, :].rearrange("p a b -> p (a b)"))
                nc.vector.tensor_add(op_sb, op_sb, xk_s[:, 0, :LT])
                nc.vector.tensor_add(op_sb, op_sb, xv_s[:, 0, :LT])
                nc.vector.tensor_add(op_sb[:, :P], op_sb[:, :P], wq_s[:, 0, :P])
                nc.vector.tensor_add(op_sb[:, :P], op_sb[:, :P], wk_s[:, 0, :P])
                nc.vector.tensor_add(op_sb[:, :P], op_sb[:, :P], wv_s[:, 0, :P])
                nc.vector.tensor_add(op_sb[:, :2 * P], op_sb[:, :2 * P], wo_s)
                for mt in range(2):
                    for lt2 in range(NLT):
                        nc.sync.dma_start(
                            out_d[mt * P:(mt + 1) * P,
                                  lt2 * LT:(lt2 + 1) * LT], op_sb)
                return

            # projected activations
            qp_s = sb.tile([P, L], bf16, name="qp_s", tag="qp_s")
            kp_s = sb.tile([P, L], bf16, name="kp_s", tag="kp_s")
            vpt_s = sb.tile([P, NMT, P], bf16, name="vpt_s", tag="vpt_s")

            # ---- projections (psum shared with attention's s_t slots) ----
            for t in range(NLT):
                for (w_s, x_s, bm_s, dst) in ((wk_s, xk_s, bkm_s, kp_s),
                                              (wq_s, xq_s, bqm_s, qp_s)):
                    ps = spp.tile([P, LT], f32, name="ps", tag="s_t")
                    nc.tensor.matmul(ps, w_s[:, 0, :],
                                     x_s[:, 0, t * LT:(t + 1) * LT],
                                     start=True, stop=False)
                    nc.tensor.matmul(ps, w_s[:, 1, :],
                                     x_s[:, 1, t * LT:(t + 1) * LT],
                                     start=False, stop=True)
                    nc.vector.tensor_tensor(
                        dst[:, t * LT:(t + 1) * LT], ps,
                        bm_s[:, t * LT:(t + 1) * LT], Alu.add)
                # v projection, transposed output: [l-chunk, ch]
                psv = spp.tile([P, LT], f32, name="psv", tag="s_t")
                for c4 in range(4):
                    c = 4 * t + c4
                    nc.tensor.matmul(psv[:, c4 * P:(c4 + 1) * P],
                                     xv_s[:, 0, c * P:(c + 1) * P],
                                     wv_s[:, 0, :], start=True, stop=False)
                    nc.tensor.matmul(psv[:, c4 * P:(c4 + 1) * P],
                                     xv_s[:, 1, c * P:(c + 1) * P],
                                     wv_s[:, 1, :], start=False, stop=True)
                nc.vector.tensor_tensor(
                    vpt_s[:, 4 * t:4 * t + 4, :], psv,
                    bvm_s[:, 4 * t:4 * t + 4, :], Alu.add)

            if variant in ("actonly", "dveonly", "race"):
                for lt in range(NLT):
                    av_ps = avp.tile([P, LT], f32, name="av_ps", tag="av_ps")
                    s_tiles = []
                    for g in range(2):
                        s_ps = spp.tile([P, 2 * LT], f32, name="s_ps",
                                        tag="s_t")
                        for hh in range(2):
                            nc.tensor.matmul(
                                s_ps[:, hh * LT:(hh + 1) * LT],
                                kp_s[hh * D:(hh + 1) * D, :P],
                                qp_s[hh * D:(hh + 1) * D, lt * LT:(lt + 1) * LT],
                                start=True, stop=True,
                                tile_position=(hh * D, 0))
                        s_tiles.append(s_ps)
                    for j in range(NMT):
                        for g in range(2):
                            pt = ptp.tile([P, 2, LT], bf16, name=f"pt{g}",
                                          tag=f"pt{g}")
                            pt_flat = pt.rearrange("p a b -> p (a b)")
                            use_dve = (variant == "dveonly" or
                                       (variant == "race" and g == 1))
                            if use_dve:
                                nc.vector.tensor_scalar(
                                    pt_flat.bitcast(i16), s_tiles[g],
                                    SCH_A, SCH_B, Alu.mult, Alu.add)
                            else:
                                nc.scalar.activation(pt_flat, s_tiles[g],
                                                     Act.Exp)
                            nc.tensor.matmul(
                                av_ps[:D, :], ones_s, pt[:, 0, :],
                                start=True, stop=True,
                                tile_position=(0, 0))
                    op_sb = epi.tile([P, LT], f32, name="op_sb", tag="op_sb")
                    nc.vector.tensor_copy(op_sb, s_ps[:, :LT])
                    nc.sync.dma_start(
                        out_d[:P, lt * LT:(lt + 1) * LT], op_sb)
                return

            # ---- attention ----
            for lt in range(NLT):
                av_ps = avp.tile([P, LT], f32, name="av_ps", tag="av_ps")
                rs_ps = rsp.tile([P, LT], f32, name="rs_ps", tag="rs_ps")
                for j in range(NMT):
                    pts = [ptp.tile([P, 2, LT], bf16, name=f"pt{g}",
                                    tag=f"pt{g}") for g in range(2)]
                    if do_scores:
                        for g in range(2):  # head pairs (0,1) and (2,3)
                            s_ps = spp.tile([P, 2 * LT], f32, name="s_ps",
                                            tag="s_t")
                            for hh in range(2):
                                h = 2 * g + hh
                                nc.tensor.matmul(
                                    s_ps[:, hh * LT:(hh + 1) * LT],
                                    kp_s[h * D:(h + 1) * D, j * P:(j + 1) * P],
                                    qp_s[h * D:(h + 1) * D, lt * LT:(lt + 1) * LT],
                                    start=True, stop=True,
                                    tile_position=(h * D, 0))
                            pt_flat = pts[g].rearrange("p a b -> p (a b)")
                            use_dve = (variant == "alldve" or
                                       (variant in ("full", "nors") and
                                        g == 1 and j in DVE_SET))
                            if variant == "allact":
                                use_dve = False
                            if use_dve:
                                nc.vector.tensor_scalar(
                                    pt_flat.bitcast(i16), s_ps,
                                    SCH_A, SCH_B, Alu.mult, Alu.add)
                            else:
                                nc.scalar.activation(pt_flat, s_ps, Act.Exp)
                    else:
                        for g in range(2):
                            nc.vector.memset(pts[g], 1.0)
                    if do_av:
                        for h in range(4):
                            nc.tensor.matmul(
                                av_ps[h * D:(h + 1) * D, :],
                                vpt_s[:, j, h * D:(h + 1) * D],
                                pts[h // 2][:, h % 2, :],
                                start=(j == 0), stop=(j == NMT - 1),
                                tile_position=(0, h * D))
                    elif do_scores:
                        # keep exps live: cheap consumers of both pt tiles
                        for h in (0, 2):
                            nc.tensor.matmul(
                                av_ps[h * D:(h + 1) * D, :],
                                vpt_s[:, j, h * D:(h + 1) * D],
                                pts[h // 2][:, h % 2, :],
                                start=(j == 0), stop=(j == NMT - 1),
                                tile_position=(0, h * D))
                    if do_rs:
                        for h in range(4):
                            nc.tensor.matmul(
                                rs_ps[h * D:(h + 1) * D, :],
                                ones_s, pts[h // 2][:, h % 2, :],
                                start=(j == 0), stop=(j == NMT - 1),
                                tile_position=(0, h * D))
                recip = epi.tile([P, LT], f32, name="recip", tag="recip")
                nc.vector.reciprocal_approx_fast(
                    recip, rs_ps if do_rs else av_ps)
                anorm = epi.tile([P, LT], bf16, name="anorm", tag="anorm")
                nc.vector.tensor_tensor(anorm, av_ps, recip, Alu.mult)
                for mt in range(2):
                    op_ps = spp.tile([P, LT], f32, name="op_ps", tag="s_t")
                    nc.tensor.matmul(op_ps, wo_s[:, mt * P:(mt + 1) * P],
                                     anorm, start=True, stop=True)
                    op_sb = epi.tile([P, LT], f32, name="op_sb", tag="op_sb")
                    nc.vector.tensor_copy(op_sb, op_ps)
                    nc.sync.dma_start(
                        out_d[mt * P:(mt + 1) * P, lt * LT:(lt + 1) * LT],
                        op_sb)

          if loop_iters > 1:
              with tc.For_i(0, loop_iters, 1):
                  body()
          else:
              for _rep in range(repeat):
                  body()

    nc.compile()
    _compiled[key] = nc
    return nc


def _prep_core_inputs(inputs):
    """Host-side folding; returns list of 8 per-core input dicts (bf16)."""
    f8 = np.float64
    bf = ml_dtypes.bfloat16
    q = np.asarray(inputs['q'], np.float32)
    k = np.asarray(inputs['k'], np.float32)
    v = np.asarray(inputs['v'], np.float32)
    pe_q = np.asarray(inputs['pe_q'], f8)
    pe_vk = np.asarray(inputs['pe_vk'], f8)

    def fold(w, b, g, beta, m, var):
        scale = np.asarray(g, f8) / np.sqrt(np.asarray(var, f8) + EPS)
        w_eff = scale[:, None] * np.asarray(w, f8)
        b_eff = (np.asarray(b, f8) - np.asarray(m, f8)) * scale + np.asarray(beta, f8)
        return w_eff, b_eff

    wq_e, bq_e = fold(inputs['wq'], inputs['bq'], inputs['gq'],
                      inputs['betaq'], inputs['mq'], inputs['varq'])
    wk_e, bk_e = fold(inputs['wk'], inputs['bk'], inputs['gk'],
                      inputs['betak'], inputs['mk'], inputs['vark'])
    wv_e, bv_e = fold(inputs['wv'], inputs['bv'], inputs['gv'],
                      inputs['betav'], inputs['mv'], inputs['varv'])
    sc = 1.0 / np.sqrt(np.float64(D))
    wq_e = wq_e * sc
    bq_e = bq_e * sc
    wo = np.asarray(inputs['wo'], f8)

    def wlayout(w_half):
        # [256 in, 128 out] -> [128 p, 2 ko, 128 m]
        return np.ascontiguousarray(
            w_half.T.reshape(2, P, P).transpose(1, 0, 2)).astype(bf)

    def xlayout(x):
        return np.ascontiguousarray(
            x.reshape(2, P, L).transpose(1, 0, 2)).astype(bf)

    half_data = []
    for hf in range(2):
        r = slice(hf * P, (hf + 1) * P)
        wq_h, wk_h, wv_h = wq_e[r], wk_e[r], wv_e[r]
        bqm = wq_h @ pe_q + bq_e[r][:, None]           # [128, L]
        bkm = wk_h @ pe_vk + bk_e[r][:, None]
        bvm = wv_h @ pe_vk + bv_e[r][:, None]
        bvmT = np.ascontiguousarray(
            bvm.T.reshape(NMT, P, P).transpose(1, 0, 2))  # [p, chunk, ch]
        half_data.append(dict(
            wq=wlayout(wq_h), wk=wlayout(wk_h), wv=wlayout(wv_h),
            wo=np.ascontiguousarray(wo[:, r].T).astype(bf),
            bqm=bqm.astype(bf), bkm=bkm.astype(bf), bvm=bvmT.astype(bf),
        ))

    in_maps = []
    for core in range(N_CORES):
        b, hf = core // 2, core % 2
        m = dict(half_data[hf])
        m['xq'] = xlayout(q[b])
        m['xk'] = xlayout(k[b])
        m['xv'] = xlayout(v[b])
        in_maps.append(m)
    return in_maps


def kernel(**inputs):
    global LAST_RESULT
    from concourse.bass_utils import run_bass_kernel_spmd

    nc = _build()
    in_maps = _prep_core_inputs(inputs)
    res = run_bass_kernel_spmd(nc, in_maps, core_ids=list(range(N_CORES)),
                               trace=PROFILE)
    LAST_RESULT = res

    q = np.asarray(inputs['q'], np.float32)
    bo = np.asarray(inputs['bo'], np.float32)
    out = np.empty((B, CQ, L), np.float32)
    for b in range(B):
        out[b] = (q[b] + bo[:, None]
                  + res.results[2 * b]['out'] + res.results[2 * b + 1]['out'])
    return out
